# revision 18
# baseline (speedup 1.0000x reference)
"""Ernie4 GQA attention layer as a Bass/Tile kernel for 8 TRN2 NeuronCores.

Sharding: core c = 4*b + g handles batch b (of 2) and head-group g (of 4).
Each group owns 8 query heads + 1 kv head (GQA 32q/4kv, head_dim 128) and the
matching column slice of w_qkv / row slice of w_o. The o_proj partial sums are
reduced on the host (all-reduce equivalent).

Per-core pipeline:
  phase 1 (token-major): qkv_tok = X^T.T @ W^T with the full 20MiB W slice
           cached in SBUF as the moving operand (N=512 f32r full rate) and
           X^T streamed exactly once as small stationary tiles.
  phase 2: per-head PE-transpose to feature-major, RoPE (swap-matmul + DVE),
           causal attention with transposed scores (scores^T[k, q]), exp on
           ACT, row-sums via a ones-matmul broadcast, PV accumulation in PSUM.
  phase 3: out_partial = ctx^T.T @ WoT (token-major psum, streamed WoT,
           tiled output layout un-tiled on host).

Feature order in wqkvt / qkv_tok scratch columns: [k, v, q0..q7].
"""
import sys

sys.path.insert(0, "/opt/trn_rl_repo")

import numpy as np

HIDDEN = 4096
N_Q_HEADS = 32
N_KV_HEADS = 4
HEAD_DIM = 128
ROPE_THETA = 500000.0
Q_SIZE = N_Q_HEADS * HEAD_DIM  # 4096
KV_SIZE = N_KV_HEADS * HEAD_DIM  # 512
B = 2
S = 2048
N_CORES = 8
N_GROUPS = 4
HEADS_PER_GROUP = N_Q_HEADS // N_GROUPS  # 8
GROUP_Q = HEADS_PER_GROUP * HEAD_DIM  # 1024
QKV_G = GROUP_Q + 2 * HEAD_DIM  # 1280 columns of qkv per group
SCALE = HEAD_DIM ** -0.5
NK = HIDDEN // 128  # 32 contraction k-tiles
NMT = QKV_G // 128  # 10 qkv feature tiles
NKT = S // 128  # 16 token/key tiles per sequence
NQB = S // 512  # 4 q-blocks
NHB = HIDDEN // 512  # 8 output-hidden blocks

_COMPILED = None
LAST_EXEC_NS = None


def _build(phases=(1, 2, 3)):
    import concourse.mybir as mybir
    import concourse.tile as tile
    from concourse import bacc

    F32 = mybir.dt.float32
    F32R = mybir.dt.float32r

    nc = bacc.Bacc("TRN2", target_bir_lowering=False, debug=False, num_devices=N_CORES)

    # xt tiled: [tt, 128, NK, 128]; xt[tt, h, ko, t] = X[tt*128+t, ko*128+h]
    # (hidden within k-tile on partitions, token within tile on free)
    xt = nc.dram_tensor("xt", [NKT, 128, NK, 128], F32R, kind="ExternalInput").ap()
    # wqkvt: [128, NK, 1280]; wqkvt[p, ko, f] = W^T[ko*128+p, f] (p = hidden)
    wqkvt = nc.dram_tensor("wqkvt", [128, NK, QKV_G], F32R, kind="ExternalInput").ap()
    wot = nc.dram_tensor("wot", [NHB, 128, HEADS_PER_GROUP, 512], F32R, kind="ExternalInput").ap()
    cos_t = nc.dram_tensor("cos_t", [HEAD_DIM, S], F32, kind="ExternalInput").ap()
    sin_t = nc.dram_tensor("sin_t", [HEAD_DIM, S], F32, kind="ExternalInput").ap()
    swp = nc.dram_tensor("swp", [128, 128], F32R, kind="ExternalInput").ap()
    ones = nc.dram_tensor("ones", [128, 128], F32R, kind="ExternalInput").ap()
    ident = nc.dram_tensor("ident", [128, 128], F32R, kind="ExternalInput").ap()
    maskt = nc.dram_tensor("maskt", [4, 128, 512], F32, kind="ExternalInput").ap()
    out_part = nc.dram_tensor(
        "out_part", [NKT, NHB, 128, 512], F32, kind="ExternalOutput"
    ).ap()

    with tile.TileContext(nc) as tc:
        with (
            tc.tile_pool(name="dram", bufs=1, space="DRAM") as dram,
        ):
            # token-major qkv scratch [tok, feat]; feature-major ctx scratch
            qkv_tok = dram.tile([S, QKV_G], F32R)
            ctx_scr = dram.tile([GROUP_Q, S], F32R)

            # ------- phase 1: qkv_tok[t, f] = sum_h X^T[h, t] W^T[h, f] -------
            with (
                tc.tile_pool(name="p1w", bufs=1) as p1w,
                tc.tile_pool(name="p1x", bufs=2) as p1x,
                tc.tile_pool(name="p1s", bufs=4) as p1s,
                tc.tile_pool(name="p1ps", bufs=5, space="PSUM") as p1ps,
            ):
                if 1 in phases:
                    w_all = p1w.tile([128, NK, QKV_G], F32R, tag="wall")
                    # chunked load so the first matmuls start early
                    for m in range(NMT):
                        nc.sync.dma_start(
                            w_all[:, :, m * 128:(m + 1) * 128],
                            wqkvt[:, :, m * 128:(m + 1) * 128],
                        )
                for tt in range(NKT if 1 in phases else 0):
                    x_tile = p1x.tile([128, NK, 128], F32R, tag="xtile")
                    nc.sync.dma_start(x_tile, xt[tt])
                    # fb blocks over features: [0:512), [512:1024), [1024:1280)
                    for fb in range(3):
                        f0 = fb * 512
                        fw = 512 if fb < 2 else 256
                        ps = p1ps.tile([128, 512], F32, tag="p1psum")
                        for k in range(NK):
                            nc.tensor.matmul(
                                ps[:, :fw],
                                x_tile[:, k, :],
                                w_all[:, k, f0:f0 + fw],
                                start=(k == 0),
                                stop=(k == NK - 1),
                            )
                        stage = p1s.tile([128, 512], F32R, tag="p1stage")
                        nc.any.tensor_copy(stage[:, :fw], ps[:, :fw])
                        nc.sync.dma_start(
                            qkv_tok[tt * 128:(tt + 1) * 128, f0:f0 + fw],
                            stage[:, :fw],
                        )

            # ------- phase 2: transpose to feature-major + RoPE + attention ---
            with (
                tc.tile_pool(name="p2c", bufs=1) as p2c,
                tc.tile_pool(name="kv", bufs=1) as kvpool,
                tc.tile_pool(name="qload", bufs=2) as qload,
                tc.tile_pool(name="qfeat", bufs=2) as qfeat,
                tc.tile_pool(name="rq", bufs=2) as rqpool,
                tc.tile_pool(name="ropet", bufs=4) as ropet,
                tc.tile_pool(name="pt", bufs=6) as ptpool,
                tc.tile_pool(name="ptm", bufs=4) as ptmpool,
                tc.tile_pool(name="rcp", bufs=3) as rcppool,
                tc.tile_pool(name="cstage", bufs=3) as cstage,
                tc.tile_pool(name="p2sc", bufs=5, space="PSUM") as p2sc,
                tc.tile_pool(name="p2r", bufs=1, space="PSUM") as p2r,
                tc.tile_pool(name="p2ctx", bufs=2, space="PSUM") as p2ctx,
            ):
                swp_sb = p2c.tile([128, 128], F32R)
                nc.sync.dma_start(swp_sb, swp)
                ones_sb = p2c.tile([128, 128], F32R)
                nc.sync.dma_start(ones_sb, ones)
                id_sb = p2c.tile([128, 128], F32R)
                nc.sync.dma_start(id_sb, ident)
                cos_sb = p2c.tile([128, S], F32)
                nc.sync.dma_start(cos_sb, cos_t)
                sin_sb = p2c.tile([128, S], F32)
                nc.sync.dma_start(sin_sb, sin_t)
                mask_sb = p2c.tile([128, 4, 512], F32)
                nc.sync.dma_start(mask_sb, maskt.rearrange("m p q -> p m q"))

                def load_tok(dst, col0):
                    # dst [128, NKT, 128] <- qkv_tok[:, col0:col0+128]
                    nc.sync.dma_start(
                        dst,
                        qkv_tok[:, col0:col0 + 128].rearrange(
                            "(tt p) d -> p tt d", p=128
                        ),
                    )

                def transpose_feat(dst, src_tok):
                    # dst [128, S] feature-major <- src_tok [128, NKT, 128]
                    for tt in range(NKT):
                        tps = p2sc.tile([128, 128], F32R, tag="scps")
                        nc.tensor.transpose(tps, src_tok[:, tt, :], id_sb)
                        nc.any.tensor_copy(dst[:, tt * 128:(tt + 1) * 128], tps)

                def rope(dst, src_tile):
                    # dst[f32r 128, S] = src*cos + (swap@src)*sin_signed
                    for c in range(S // 512):
                        cs = slice(c * 512, (c + 1) * 512)
                        sw_ps = p2sc.tile([128, 512], F32, tag="scps")
                        nc.tensor.matmul(
                            sw_ps, swp_sb, src_tile[:, cs], start=True, stop=True
                        )
                        t_sin = ropet.tile([128, 512], F32, tag="tsin")
                        nc.vector.tensor_mul(t_sin, sw_ps, sin_sb[:, cs])
                        t_cos = ropet.tile([128, 512], F32, tag="tcos")
                        nc.vector.tensor_mul(
                            t_cos, src_tile[:, cs].bitcast(F32), cos_sb[:, cs]
                        )
                        nc.vector.tensor_add(dst[:, cs], t_cos, t_sin)

                if 2 in phases:
                    k_tok = kvpool.tile([128, NKT, 128], F32R, tag="ktok")
                    load_tok(k_tok, 0)
                    k_feat = kvpool.tile([128, S], F32R, tag="kfeat")
                    transpose_feat(k_feat, k_tok)
                    rk = kvpool.tile([128, S], F32R, tag="rk")
                    rope(rk, k_feat)

                    vtok = kvpool.tile([128, NKT, 128], F32R, tag="vtok")
                    load_tok(vtok, 128)

                for h in range(HEADS_PER_GROUP if 2 in phases else 0):
                    q_tok = qload.tile([128, NKT, 128], F32R, tag="qtok")
                    load_tok(q_tok, 256 + h * 128)
                    qh = qfeat.tile([128, S], F32R, tag="qh")
                    transpose_feat(qh, q_tok)
                    rq = rqpool.tile([128, S], F32R, tag="rq")
                    rope(rq, qh)

                    for j in range(NQB):
                        qs = slice(j * 512, (j + 1) * 512)
                        nkt_j = 4 * (j + 1)  # causal: k-tiles 0..4j+3
                        ctx_ps = p2ctx.tile([128, 512], F32, tag="ctxps")
                        r_ps = p2r.tile([128, 512], F32, tag="rps")
                        for kt in range(nkt_j):
                            sc_ps = p2sc.tile([128, 512], F32, tag="scps")
                            nc.tensor.matmul(
                                sc_ps,
                                rk[:, kt * 128:(kt + 1) * 128],
                                rq[:, qs],
                                start=True,
                                stop=True,
                            )
                            pt = ptpool.tile([128, 512], F32R, tag="pt")
                            nc.scalar.activation(
                                pt, sc_ps,
                                mybir.ActivationFunctionType.Exp,
                                scale=SCALE,
                            )
                            if kt >= 4 * j:  # diagonal tile: causal mask
                                di = kt - 4 * j
                                ptm = ptmpool.tile([128, 512], F32R, tag="ptm")
                                nc.vector.tensor_mul(
                                    ptm, pt.bitcast(F32), mask_sb[:, di, :]
                                )
                                pt_use = ptm
                            else:
                                pt_use = pt
                            nc.tensor.matmul(
                                r_ps, ones_sb, pt_use,
                                start=(kt == 0), stop=(kt == nkt_j - 1),
                            )
                            nc.tensor.matmul(
                                ctx_ps, vtok[:, kt, :], pt_use,
                                start=(kt == 0), stop=(kt == nkt_j - 1),
                            )
                        rcp = rcppool.tile([128, 512], F32, tag="rcp")
                        nc.vector.reciprocal(rcp, r_ps)
                        cst = cstage.tile([128, 512], F32R, tag="cst")
                        nc.vector.tensor_mul(cst, ctx_ps, rcp)
                        nc.sync.dma_start(
                            ctx_scr[h * 128:(h + 1) * 128, qs], cst
                        )

            # ------- phase 3: out = ctx^T.T @ WoT -----------------------------
            with (
                tc.tile_pool(name="p3ctx", bufs=1) as p3ctx,
                tc.tile_pool(name="p3w", bufs=3) as p3w,
                tc.tile_pool(name="p3s", bufs=4) as p3s,
                tc.tile_pool(name="p3ps", bufs=4, space="PSUM") as p3ps,
            ):
                ctx_sb = p3ctx.tile([128, HEADS_PER_GROUP, S], F32R)
                for hh in range(HEADS_PER_GROUP):
                    nc.sync.dma_start(
                        ctx_sb[:, hh, :], ctx_scr[hh * 128:(hh + 1) * 128, :]
                    )
                for hb in range(NHB if 3 in phases else 0):
                    wo_blk = p3w.tile([128, HEADS_PER_GROUP, 512], F32R, tag="woblk")
                    nc.sync.dma_start(wo_blk, wot[hb])
                    for tt in range(NKT):
                        ps = p3ps.tile([128, 512], F32, tag="p3psum")
                        for hk in range(HEADS_PER_GROUP):
                            nc.tensor.matmul(
                                ps,
                                ctx_sb[:, hk, tt * 128:(tt + 1) * 128],
                                wo_blk[:, hk, :],
                                start=(hk == 0),
                                stop=(hk == HEADS_PER_GROUP - 1),
                            )
                        stage = p3s.tile([128, 512], F32, tag="p3stage")
                        nc.any.tensor_copy(stage, ps)
                        nc.sync.dma_start(out_part[tt, hb], stage)

    nc.compile()
    return nc


def _host_inputs(positions, hidden_states, w_qkv, w_o):
    """Shard + lay out inputs for the 8 cores (c = 4*b + g)."""
    positions = np.asarray(positions)
    hidden_states = np.asarray(hidden_states, dtype=np.float32)
    w_qkv = np.asarray(w_qkv, dtype=np.float32)
    w_o = np.asarray(w_o, dtype=np.float32)

    inv_freq = 1.0 / (ROPE_THETA ** (np.arange(0, HEAD_DIM, 2, dtype=np.float64) / HEAD_DIM))
    ang = positions.astype(np.float64)[None, :] * inv_freq[:, None]  # [half, S]
    cos_t = np.empty((HEAD_DIM, S), dtype=np.float32)
    sin_t = np.empty((HEAD_DIM, S), dtype=np.float32)
    c = np.cos(ang).astype(np.float32)
    s = np.sin(ang).astype(np.float32)
    cos_t[0::2] = c
    cos_t[1::2] = c
    sin_t[0::2] = -s
    sin_t[1::2] = s

    swp = np.zeros((128, 128), dtype=np.float32)
    idx = np.arange(0, 128, 2)
    swp[idx, idx + 1] = 1.0
    swp[idx + 1, idx] = 1.0
    ones = np.ones((128, 128), dtype=np.float32)
    ident = np.eye(128, dtype=np.float32)

    q_loc = np.arange(512)[None, :]
    k_loc = np.arange(128)[:, None]
    maskt = np.stack(
        [(q_loc - k_loc - 128 * di >= 0).astype(np.float32) for di in range(4)]
    )  # [4, 128, 512]

    # xt tiled: [NKT, 128, NK, 128]; xt_t[tt, hh, ko, t] = X[tt*128+t, ko*128+hh]
    # (hidden on partitions: lhsT tiles for the token-major qkv matmul)
    xts = []
    for b in range(B):
        xt_t = np.ascontiguousarray(
            hidden_states[b].reshape(NKT, 128, NK, 128).transpose(0, 3, 2, 1)
        )
        xts.append(xt_t)

    in_maps = []
    for c_id in range(N_CORES):
        b, g = divmod(c_id, N_GROUPS)
        cols = np.concatenate([
            np.arange(Q_SIZE + g * HEAD_DIM, Q_SIZE + (g + 1) * HEAD_DIM),  # k
            np.arange(Q_SIZE + KV_SIZE + g * HEAD_DIM, Q_SIZE + KV_SIZE + (g + 1) * HEAD_DIM),  # v
            np.arange(g * GROUP_Q, (g + 1) * GROUP_Q),  # q0..q7
        ])
        # wqkvt: [128, NK, 1280]; [p, ko, f] = w_qkv[cols[f], ko*128+p]
        wq = w_qkv[cols, :]  # [1280, 4096]
        wqkvt_t = np.ascontiguousarray(
            wq.T.reshape(NK, 128, QKV_G).transpose(1, 0, 2)
        )
        wot_full = w_o[:, g * GROUP_Q:(g + 1) * GROUP_Q].T  # [1024, 4096]
        wot_t = np.ascontiguousarray(
            wot_full.reshape(HEADS_PER_GROUP, 128, NHB, 512).transpose(2, 1, 0, 3)
        )
        in_maps.append({
            "xt": xts[b],
            "wqkvt": wqkvt_t,
            "wot": wot_t,
            "cos_t": cos_t,
            "sin_t": sin_t,
            "swp": swp,
            "ones": ones,
            "ident": ident,
            "maskt": maskt,
        })
    return in_maps


def kernel(positions, hidden_states, w_qkv, w_o):
    global _COMPILED, LAST_EXEC_NS
    from concourse import bass_utils

    if _COMPILED is None:
        _COMPILED = _build()
    nc = _COMPILED

    in_maps = _host_inputs(positions, hidden_states, w_qkv, w_o)
    res = bass_utils.run_bass_kernel_spmd(
        nc, in_maps, core_ids=list(range(N_CORES))
    )
    LAST_EXEC_NS = res.exec_time_ns

    out = np.zeros((B, S, HIDDEN), dtype=np.float32)
    for c_id in range(N_CORES):
        b = c_id // N_GROUPS
        part = res.results[c_id]["out_part"]  # [NKT, NHB, 128, 512]
        out[b] += part.transpose(0, 2, 1, 3).reshape(S, HIDDEN)
    return out
